# revision 5
# baseline (speedup 1.0000x reference)
"""AttentionLSTM Trainium2 kernel, v3: 8-way tensor parallel recurrence.

Each core owns 512 of the 4096 gate rows (the 4 gates g,i,f,o of a 128-dim
h-slice) for the FULL batch of 64. Per timestep a core computes its h-chunk
[128, 64] and broadcasts it to all 8 cores via remote_dma_broadcast with a
partition_id-indexed slot (sigma-independent); the next step's matmuls
consume the assembled h [8k x 64b] from the receive ring.

This cuts the per-step LDWEIGHTS wall 8x vs data-parallel (32 weight tiles
per step instead of 256), which is the dominant cost at batch-8-per-core.

Phases:
  P1 (Tile) L0 input projection from full-batch embeddings -> xp SBUF
  R0 (raw)  L0 recurrence, 256 steps, h -> h1seq DRAM
  P3 (Tile) L1 input projection from h1seq -> xp SBUF
  R1 (raw)  L1 recurrence; per-step DVE extracts this core's batch slice
            into h2sel (t-major, 8b per t — same layout the attention uses)
  P5 (Tile) additive attention + head for this core's 8 batch rows
"""

import os
import numpy as np
from contextlib import ExitStack

B, T, V, E, H, A, C = 64, 256, 32000, 512, 1024, 512, 2
NCORES = 8
BL = 8            # batch rows per core for the attention phase
FB = 64           # full batch (recurrence free dim)
MT = 4            # m-tiles per core (4 gates x 128 rows)
KT = 8            # k-tiles over H
KT0 = 4           # k-tiles over E
GPERM = [2, 0, 1, 3]   # gate row reorder: [g, i, f, o]
SEQ = (T + 1) * BL
RING = 4

_CACHE = {}


def _bf16(a):
    import ml_dtypes
    return np.ascontiguousarray(np.asarray(a, np.float32)).astype(ml_dtypes.bfloat16)


def _f32(a):
    return np.ascontiguousarray(a, np.float32)


def _tiles(w, mt, kt):
    m4 = w.reshape(mt, 128, kt, 128)
    return np.ascontiguousarray(np.transpose(m4, (3, 0, 2, 1))).reshape(
        128, mt * kt, 128
    )


def _gate_reorder(w):
    g = w.reshape(4, H, -1)
    return np.concatenate([g[p] for p in GPERM], 0)


def _core_rows(wr, j):
    """wr: [4H, D] gate-reordered. Core j's 512 rows: gates x h-slice j."""
    return np.concatenate(
        [wr[g * H + 128 * j : g * H + 128 * (j + 1)] for g in range(4)], 0
    )


def _prep_shared(inputs):
    wkeys = ("w_ih0", "w_hh0", "b_ih0", "b_hh0", "w_ih1", "w_hh1",
             "b_ih1", "b_hh1", "m1_w", "m1_b", "m2_w", "m2_b", "v",
             "n_w", "n_b", "out_w", "out_b")
    key = tuple(id(inputs[k]) for k in wkeys)
    hit = _CACHE.get("sh")
    if hit is not None and hit[0] == key:
        return hit[1]
    sh = {"percore": []}
    for L, (wi, wh, bi, bh, kt) in {
        0: ("w_ih0", "w_hh0", "b_ih0", "b_hh0", KT0),
        1: ("w_ih1", "w_hh1", "b_ih1", "b_hh1", KT),
    }.items():
        sh[f"wir{L}"] = _gate_reorder(_f32(inputs[wi]))
        sh[f"whr{L}"] = _gate_reorder(_f32(inputs[wh]))
        sh[f"br{L}"] = _gate_reorder(
            (_f32(inputs[bi]) + _f32(inputs[bh]))[:, None]
        )[:, 0]
    for j in range(NCORES):
        d = {}
        for L, kt in ((0, KT0), (1, KT)):
            d[f"wi{L}"] = _bf16(_tiles(_core_rows(sh[f"wir{L}"], j), MT, kt))
            d[f"wh{L}"] = _bf16(_tiles(_core_rows(sh[f"whr{L}"], j), MT, KT))
            d[f"b{L}"] = _f32(
                _core_rows(sh[f"br{L}"][:, None], j).reshape(MT, 128).T
            )  # [128, 4]
        sh["percore"].append(d)

    sh["m1w"] = _bf16(_tiles(_f32(inputs["m1_w"]), 4, KT))
    sh["m2w"] = _bf16(_tiles(_f32(inputs["m2_w"]), 4, KT))
    sh["m1b"] = _f32(_f32(inputs["m1_b"]).reshape(4, 128).T)
    sh["m2b"] = _f32(_f32(inputs["m2_b"]).reshape(4, 128).T)
    sh["vT"] = _bf16(_f32(inputs["v"]).reshape(4, 128).T)
    sh["nw"] = _bf16(_tiles(_f32(inputs["n_w"]), 8, 16))
    sh["nb"] = _f32(_f32(inputs["n_b"]).reshape(8, 128).T)
    ow = _f32(inputs["out_w"]).T
    sh["ow"] = _bf16(ow.reshape(8, 128, 2).transpose(1, 0, 2))
    sh["ob"] = _f32(_f32(inputs["out_b"]).reshape(2, 1))
    _CACHE["sh"] = (key, sh)
    return sh


def _install_drain_patch():
    from concourse.tile import TileContext, ScopedClock

    if getattr(TileContext, "_drain_patched", False):
        return

    def _patched(self, tick_clock, wait_clock):
        drain_inst = self.nc.sync.drain()
        wait_clock.add_sem_waits(
            drain_inst.ins, ScopedClock({None: tick_clock.global_clock})
        )
        si = drain_inst.ins.sync_info
        waits = list(si.on_wait)
        if len(waits) > 1:
            si.on_wait = waits[:1]
            for w in waits[1:]:
                d2 = self.nc.sync.drain()
                wait_clock.add_sem_waits(
                    d2.ins, ScopedClock({None: tick_clock.global_clock})
                )
                d2.ins.sync_info.on_wait = [w]
        self.nc.all_engine_barrier()
        popped = self.nc._tile_sem_poison_stack.pop()
        assert popped is self._sem_poison
        self.nc.clear_and_free_semaphores(list(self.sems.allocated().values()))
        self.nc.all_engine_barrier()

    TileContext._drain_and_barrier = _patched
    TileContext._drain_patched = True


def _split_waits(nc, limit=1):
    import copy
    import concourse.mybir as mybir

    n_split = 0
    for f in nc.m.functions:
        for bb in f.blocks:
            li = list(bb.instructions)
            out = []
            for inst in li:
                si = inst.sync_info
                if si is not None and len(si.on_wait) > limit:
                    waits = list(si.on_wait)
                    for j in range(0, len(waits) - limit, limit):
                        nop = mybir.InstNoOp(
                            name=f"{inst.name}_ws{j}", ins=[], outs=[]
                        )
                        nop.engine = inst.engine
                        si2 = copy.copy(si)
                        si2.on_wait = waits[j : j + limit]
                        si2.on_update = []
                        nop.sync_info = si2
                        out.append(nop)
                        n_split += 1
                    si.on_wait = waits[len(waits) - limit :]
                inst.sync_info = si
                out.append(inst)
            bb.instructions = out
    return n_split


def _build_program():
    import concourse.bass as bass
    import concourse.mybir as mybir
    from concourse.bass import ds
    from concourse.tile import TileContext
    from concourse import library_config
    from concourse.library_overlay import lower_extended_insts

    _install_drain_patch()

    dt = mybir.dt
    AF = mybir.ActivationFunctionType
    OP = mybir.AluOpType
    AX = mybir.AxisListType

    nc = bass.Bass(num_devices=NCORES)

    # ---- DRAM I/O ----
    embT_d = nc.dram_tensor("embT", [128, KT0, T * FB], dt.bfloat16, kind="ExternalInput")
    wi0_d = nc.dram_tensor("wi0", [128, MT * KT0, 128], dt.bfloat16, kind="ExternalInput")
    wh0_d = nc.dram_tensor("wh0", [128, MT * KT, 128], dt.bfloat16, kind="ExternalInput")
    wi1_d = nc.dram_tensor("wi1", [128, MT * KT, 128], dt.bfloat16, kind="ExternalInput")
    wh1_d = nc.dram_tensor("wh1", [128, MT * KT, 128], dt.bfloat16, kind="ExternalInput")
    b0_d = nc.dram_tensor("b0", [128, MT], dt.float32, kind="ExternalInput")
    b1_d = nc.dram_tensor("b1", [128, MT], dt.float32, kind="ExternalInput")
    m1w_d = nc.dram_tensor("m1w", [128, 4 * KT, 128], dt.bfloat16, kind="ExternalInput")
    m2w_d = nc.dram_tensor("m2w", [128, 4 * KT, 128], dt.bfloat16, kind="ExternalInput")
    m1b_d = nc.dram_tensor("m1b", [128, 4], dt.float32, kind="ExternalInput")
    m2b_d = nc.dram_tensor("m2b", [128, 4], dt.float32, kind="ExternalInput")
    vT_d = nc.dram_tensor("vT", [128, 4], dt.bfloat16, kind="ExternalInput")
    nw_d = nc.dram_tensor("nw", [128, 8 * 16, 128], dt.bfloat16, kind="ExternalInput")
    nb_d = nc.dram_tensor("nb", [128, 8], dt.float32, kind="ExternalInput")
    ow_d = nc.dram_tensor("ow", [128, 8, 2], dt.bfloat16, kind="ExternalInput")
    ob_d = nc.dram_tensor("ob", [2, 1], dt.float32, kind="ExternalInput")
    out_d = nc.dram_tensor("out_t", [2, BL], dt.float32, kind="ExternalOutput")
    h1seq_d = nc.dram_tensor(
        "h1seq", [128, T, KT * FB], dt.bfloat16,
        kind=("ExternalOutput" if os.environ.get("TP8_DEBUG_H1") else "Internal"),
    )
    xpdump_d = (
        nc.dram_tensor("xpdump", [128, MT, T * FB], dt.bfloat16, kind="ExternalOutput")
        if os.environ.get("TP8_DEBUG_XP")
        else None
    )

    st = ExitStack()
    # ---- persistent SBUF ----
    wh = st.enter_context(nc.sbuf_tensor("wh", [128, MT * KT, 128], dt.bfloat16))
    recv = st.enter_context(nc.sbuf_tensor("recv", [128, RING, KT * FB], dt.bfloat16))
    send = st.enter_context(nc.sbuf_tensor("send", [128, FB], dt.bfloat16))
    h2sel = st.enter_context(nc.sbuf_tensor("h2sel", [128, KT, SEQ], dt.bfloat16))
    cst = st.enter_context(nc.sbuf_tensor("cst", [128, FB], dt.float32))
    gsum = st.enter_context(nc.sbuf_tensor("gsum", [128, 4, FB], dt.float32))
    gact = st.enter_context(nc.sbuf_tensor("gact", [128, 4, FB], dt.float32))
    t1 = st.enter_context(nc.sbuf_tensor("t1", [128, FB], dt.float32))
    c2 = st.enter_context(nc.sbuf_tensor("c2", [128, FB], dt.float32))
    tcs = st.enter_context(nc.sbuf_tensor("tcs", [128, FB], dt.float32))
    dummy = st.enter_context(nc.sbuf_tensor("dmy_s", [128, 1], dt.float32))
    # xp is huge (128KB/partition); allocated on its own stack so it can be
    # freed before the attention phase needs the space.
    stx = ExitStack()
    xp = stx.enter_context(nc.sbuf_tensor("xp", [128, MT, T * FB], dt.bfloat16))

    # ---- semaphores (raw sections) ----
    sem = {}
    for L in range(2):
        for nm in ("lsem", "psem", "hsem", "mmsem", "actsem",
                   "dvasem", "dvcsem"):
            sem[f"{nm}{L}"] = nc.alloc_semaphore(f"{nm}{L}")
        sem[f"rsems{L}"] = [
            nc.alloc_semaphore(f"rsem{L}_{q}") for q in range(RING)
        ]
    whdsem = nc.alloc_semaphore("whdsem")
    h1dsem = nc.alloc_semaphore("h1dsem")

    # gpsimd prologue: comm library + identity registers
    nc.gpsimd.load_library(library_config.remote_dma)
    pid_g = nc.gpsimd.partition_id()
    pid_v = nc.vector.partition_id()
    pg64 = pid_g * FB
    pv8 = pid_v * BL

    # initial wh0 load (SP)
    nc.sync.dma_start(wh[:], wh0_d[:]).then_inc(whdsem, 16)

    # =========== P1: L0 input projection (Tile) ===========
    with TileContext(nc) as tc:
        with (
            tc.tile_pool(name="p1", bufs=1) as p1,
            tc.tile_pool(name="emb", bufs=3) as ep,
            tc.tile_pool(name="pp1", bufs=4, space="PSUM") as pp1,
        ):
            wi0 = p1.tile([128, MT * KT0, 128], dt.bfloat16)
            nc.sync.dma_start(wi0[:], wi0_d[:])
            b0s = p1.tile([128, MT], dt.float32)
            nc.sync.dma_start(b0s[:], b0_d[:])
            nc.vector.memset(h2sel[:, :, 0:BL], 0.0)
            nc.vector.memset(cst[:], 0.0)
            for nn in range(32):
                eb = ep.tile([128, KT0, 512], dt.bfloat16, tag="eb")
                nc.sync.dma_start(eb[:], embT_d[:, :, nn * 512 : (nn + 1) * 512])
                for m in range(MT):
                    ps = pp1.tile([128, 512], dt.float32, tag="mm")
                    for k in range(KT0):
                        nc.tensor.matmul(
                            ps[:],
                            wi0[:, m * KT0 + k, :],
                            eb[:, k, :],
                            start=(k == 0),
                            stop=(k == KT0 - 1),
                        )
                    nc.scalar.activation(
                        xp[:, m, nn * 512 : (nn + 1) * 512],
                        ps[:],
                        AF.Identity,
                        bias=b0s[:, m : m + 1],
                    )

    # =========== recurrence (raw) ===========
    ctr = dict()

    def inc(s):
        ctr[s] = ctr.get(s, 0) + 1
        return ctr[s]

    def recurrence(L, extract):
        rsems, lsem, psem = sem[f"rsems{L}"], sem[f"lsem{L}"], sem[f"psem{L}"]

        def rwait(eng, rnd):
            # arrivals of round `rnd` complete: its ring-slot sem counts
            # 16 per use of that slot
            eng.wait_ge(rsems[rnd % RING], 16 * (rnd // RING + 1))
        hsem, mmsem = sem[f"hsem{L}"], sem[f"mmsem{L}"]
        actsem, dvasem, dvcsem = (
            sem[f"actsem{L}"], sem[f"dvasem{L}"], sem[f"dvcsem{L}"],
        )
        rst = ExitStack()
        PS = [
            rst.enter_context(
                nc.psum_tensor(f"ps{L}_{m}", [128, FB], dt.float32)
            )
            for m in range(4)
        ]
        # drain scratch: the per-gate "done" sem is deferred to the NEXT
        # group's stop matmul so the PSUM writeback pipeline (128-cycle
        # drain) is provably flushed before the DVE reads the bank. The
        # last gate is covered by a 3-matmul dummy group into PSX.
        PSX = rst.enter_context(
            nc.psum_tensor(f"psx{L}", [128, FB], dt.float32)
        )

        # keep both ACT tables resident before the loop
        nc.scalar.activation(dummy[:], dummy[:], AF.Tanh)
        nc.scalar.activation(dummy[:], dummy[:], AF.Sigmoid)

        def gen_desc(r):
            nc.gpsimd.remote_dma_broadcast(
                recv[:, r % RING, :][:, ds(pg64, FB)],
                send[:],
                remote_sem=rsems[r % RING],
                local_sem=lsem,
                rdests=[(0, k) for k in range(NCORES)],
            ).then_inc(psem, 1)
            inc(psem.name)

        def xs(m, r):
            return xp[:, m, r * FB : (r + 1) * FB]

        # ---- step 0: gates = xs (h_{-1}=0); c0 = sig(i)*tanh(g) ----
        a_tg = inc(actsem.name)
        nc.scalar.activation(gact[:, 0, :], xs(0, 0), AF.Tanh).then_inc(actsem, 1)
        a_si = inc(actsem.name)
        nc.scalar.activation(gact[:, 1, :], xs(1, 0), AF.Sigmoid).then_inc(actsem, 1)
        a_so = inc(actsem.name)
        nc.scalar.activation(gact[:, 3, :], xs(3, 0), AF.Sigmoid).then_inc(actsem, 1)
        nc.vector.wait_ge(actsem, a_si)
        nc.vector.tensor_tensor(
            out=cst[:], in0=gact[:, 1, :], in1=gact[:, 0, :], op=OP.mult
        ).then_inc(dvcsem, 1)
        inc(dvcsem.name)
        a_tc = inc(actsem.name)
        nc.scalar.wait_ge(dvcsem, ctr[dvcsem.name])
        nc.scalar.activation(tcs[:], cst[:], AF.Tanh).then_inc(actsem, 1)
        nc.vector.wait_ge(actsem, a_tc)
        nc.vector.tensor_tensor(
            out=send[:], in0=gact[:, 3, :], in1=tcs[:], op=OP.mult
        ).then_inc(hsem, 1)
        inc(hsem.name)
        gen_desc(0)

        # gpsimd comm schedule for all rounds
        for r in range(T):
            # psem first (satisfied a round in advance); final wake on hsem
            nc.gpsimd.wait_ge(psem, r + 1)
            nc.gpsimd.wait_ge(hsem, r + 1)
            nc.gpsimd.trigger_dma(count=1)
            if r + 1 < T:
                gen_desc(r + 1)

        # ---- steps 1..T-1 ----
        for r in range(1, T):
            rv = recv[:, (r - 1) % RING, :]
            # PE
            rwait(nc.tensor, r - 1)
            for m in range(4):
                for k in range(KT):
                    mm = nc.tensor.matmul(
                        PS[m][:],
                        wh[:, m * KT + k, :],
                        rv[:, k * FB : (k + 1) * FB],
                        start=(k == 0),
                        stop=(k == KT - 1),
                    )
                if m >= 1:
                    # group m's stop also certifies group m-1's writeback
                    mm.then_inc(mmsem, 1)
                    inc(mmsem.name)
            for k in range(3):
                mm = nc.tensor.matmul(
                    PSX[:],
                    wh[:, 3 * KT + KT - 1, :],
                    rv[:, (KT - 1) * FB : KT * FB],
                    start=(k == 0),
                    stop=(k == 2),
                )
            mm.then_inc(mmsem, 1)
            inc(mmsem.name)
            mmbase = ctr[mmsem.name] - 4

            # DVE: extraction of round r-1 data (L1 only)
            if extract:
                rwait(nc.vector, r - 1)
                nc.vector.tensor_copy(
                    h2sel[:, :, r * BL : (r + 1) * BL],
                    rv.rearrange("p (k b) -> p k b", b=FB)[:, :, ds(pv8, BL)],
                )

            # DVE adds (gate = psum + xs)
            for m in range(4):
                nc.vector.wait_ge(mmsem, mmbase + m + 1)
                nc.vector.tensor_tensor(
                    out=gsum[:, m, :], in0=PS[m][:], in1=xs(m, r), op=OP.add
                ).then_inc(dvasem, inc(dvasem.name))
            dvabase = ctr[dvasem.name] - 4

            # ACT nonlinearities
            acts = []
            for m, f in ((0, AF.Tanh), (1, AF.Sigmoid), (2, AF.Sigmoid),
                         (3, AF.Sigmoid)):
                nc.scalar.wait_ge(dvasem, dvabase + m + 1)
                nc.scalar.activation(gact[:, m, :], gsum[:, m, :], f).then_inc(
                    actsem, 1
                )
                acts.append(inc(actsem.name))

            # DVE cell chain
            nc.vector.wait_ge(actsem, acts[1])
            nc.vector.tensor_tensor(
                out=t1[:], in0=gact[:, 1, :], in1=gact[:, 0, :], op=OP.mult
            )
            nc.vector.wait_ge(actsem, acts[2])
            nc.vector.tensor_tensor(
                out=c2[:], in0=gact[:, 2, :], in1=cst[:], op=OP.mult
            )
            nc.vector.tensor_tensor(
                out=cst[:], in0=c2[:], in1=t1[:], op=OP.add
            ).then_inc(dvcsem, 1)
            inc(dvcsem.name)
            a_tc = inc(actsem.name)
            nc.scalar.wait_ge(dvcsem, ctr[dvcsem.name])
            nc.scalar.activation(tcs[:], cst[:], AF.Tanh).then_inc(actsem, 1)
            # h = sig(o) * tanh(c)  -> send buffer (bf16)
            # (no lsem guard needed: this step's arrival gate implies the
            # previous round's broadcast finished reading `send`)
            nc.vector.wait_ge(actsem, a_tc)
            nc.vector.tensor_tensor(
                out=send[:], in0=gact[:, 3, :], in1=tcs[:], op=OP.mult
            ).then_inc(hsem, 1)
            inc(hsem.name)

            # SP: stream assembled h of round r-1 out (L0 only)
            if not extract:
                rwait(nc.sync, r - 1)
                nc.sync.dma_start(h1seq_d[:, r - 1, :], rv[:]).then_inc(h1dsem, 16)
                inc("h1d")

        # tail: last round's assembled h
        if extract:
            rwait(nc.vector, T - 1)
            nc.vector.tensor_copy(
                h2sel[:, :, T * BL : (T + 1) * BL],
                recv[:, (T - 1) % RING, :].rearrange("p (k b) -> p k b", b=FB)[
                    :, :, ds(pv8, BL)
                ],
            )
        else:
            rwait(nc.sync, T - 1)
            nc.sync.dma_start(
                h1seq_d[:, T - 1, :], recv[:, (T - 1) % RING, :]
            ).then_inc(h1dsem, 16)
            inc("h1d")
            nc.sync.wait_ge(h1dsem, 16 * ctr["h1d"])
        rst.close()

    if xpdump_d is not None:
        xpd_sem = nc.alloc_semaphore("xpd_sem")
        nc.sync.dma_start(xpdump_d[:], xp[:]).then_inc(xpd_sem, 16)
        nc.sync.wait_ge(xpd_sem, 16)
        nc.all_engine_barrier()

    # ---- R0: L0 recurrence ----
    nc.tensor.wait_ge(whdsem, 16)
    recurrence(0, extract=False)
    nc.all_engine_barrier()

    # load wh1 over wh
    nc.sync.dma_start(wh[:], wh1_d[:]).then_inc(whdsem, 16)

    # =========== P3: L1 input projection (Tile) ===========
    with TileContext(nc) as tc:
        with (
            tc.tile_pool(name="p3", bufs=1) as p3,
            tc.tile_pool(name="h1b", bufs=2) as hp,
            tc.tile_pool(name="pp3", bufs=4, space="PSUM") as pp3,
        ):
            wi1 = p3.tile([128, MT * KT, 128], dt.bfloat16)
            nc.sync.dma_start(wi1[:], wi1_d[:])
            b1s = p3.tile([128, MT], dt.float32)
            nc.sync.dma_start(b1s[:], b1_d[:])
            for nn in range(32):
                hb = hp.tile([128, 8, KT * FB], dt.bfloat16, tag="hb")
                nc.sync.dma_start(hb[:], h1seq_d[:, nn * 8 : (nn + 1) * 8, :])
                for m in range(MT):
                    ps = pp3.tile([128, 512], dt.float32, tag="mm")
                    psv = ps[:].rearrange("p (t b) -> p t b", b=FB)
                    for k in range(KT):
                        nc.tensor.matmul(
                            psv,
                            wi1[:, m * KT + k, :],
                            hb[:, :, k * FB : (k + 1) * FB],
                            start=(k == 0),
                            stop=(k == KT - 1),
                        )
                    nc.scalar.activation(
                        xp[:, m, nn * 512 : (nn + 1) * 512],
                        ps[:],
                        AF.Identity,
                        bias=b1s[:, m : m + 1],
                    )

    # ---- R1: L1 recurrence ----
    nc.tensor.wait_ge(whdsem, 32)
    recurrence(1, extract=True)
    nc.all_engine_barrier()
    stx.close()

    # =========== P5: attention + head (Tile) ===========
    h_seq = h2sel
    with TileContext(nc) as tc:
        with (
            tc.tile_pool(name="att", bufs=1) as at,
            tc.tile_pool(name="attm", bufs=2) as atm,
            tc.tile_pool(name="ap1", bufs=2, space="PSUM") as ap1,
            tc.tile_pool(name="ap2", bufs=2, space="PSUM") as ap2,
            tc.tile_pool(name="ap3", bufs=1, space="PSUM") as ap3,
        ):
            m1w = at.tile([128, 4 * KT, 128], dt.bfloat16)
            nc.sync.dma_start(m1w[:], m1w_d[:])
            m2w = at.tile([128, 4 * KT, 128], dt.bfloat16)
            nc.sync.dma_start(m2w[:], m2w_d[:])
            m1b = at.tile([128, 4], dt.float32)
            nc.sync.dma_start(m1b[:], m1b_d[:])
            m2b = at.tile([128, 4], dt.float32)
            nc.sync.dma_start(m2b[:], m2b_d[:])
            vT = at.tile([128, 4], dt.bfloat16)
            nc.sync.dma_start(vT[:], vT_d[:])
            nw = at.tile([128, 8 * 16, 128], dt.bfloat16)
            nc.sync.dma_start(nw[:], nw_d[:])
            nb = at.tile([128, 8], dt.float32)
            nc.sync.dma_start(nb[:], nb_d[:])
            ow = at.tile([128, 8, 2], dt.bfloat16)
            nc.sync.dma_start(ow[:], ow_d[:])
            ob = at.tile([2, 1], dt.float32)
            nc.sync.dma_start(ob[:], ob_d[:])
            ones = at.tile([1, 128], dt.bfloat16)
            nc.vector.memset(ones[:], 1.0)

            hl = T * BL
            hv = h_seq[:, :, BL:SEQ]

            m2T = at.tile([128, 4, BL], dt.float32)
            for m in range(4):
                ps = ap2.tile([128, BL], dt.float32, tag="sm")
                for k in range(KT):
                    nc.tensor.matmul(
                        ps[:],
                        m2w[:, m * KT + k, :],
                        h_seq[:, k, hl : hl + BL],
                        start=(k == 0),
                        stop=(k == KT - 1),
                    )
                nc.scalar.activation(
                    m2T[:, m, :], ps[:], AF.Identity, bias=m2b[:, m : m + 1]
                )

            u = at.tile([128, 4, 2048], dt.bfloat16)
            tmpu = atm.tile([128, 512], dt.float32, tag="tmpu")
            for m in range(4):
                for nn in range(4):
                    t0 = nn * 64
                    ps = ap1.tile([128, 512], dt.float32, tag="big")
                    psv = ps[:].rearrange("p (b t) -> p b t", t=64)
                    for k in range(KT):
                        rhs = (
                            hv[:, k, :]
                            .rearrange("p (t b) -> p b t", b=8)[
                                :, :, t0 : t0 + 64
                            ]
                        )
                        nc.tensor.matmul(
                            psv,
                            m1w[:, m * KT + k, :],
                            rhs,
                            start=(k == 0),
                            stop=(k == KT - 1),
                        )
                    tv = tmpu[:].rearrange("p (b t) -> p b t", t=64)
                    nc.vector.tensor_tensor(
                        out=tv,
                        in0=psv,
                        in1=m2T[:, m, :].to_broadcast([128, 8, 64]),
                        op=OP.add,
                    )
                    uv = u[:, m, :].rearrange("p (b t) -> p b t", t=256)[
                        :, :, t0 : t0 + 64
                    ]
                    nc.scalar.activation(
                        uv, tv, AF.Tanh, bias=m1b[:, m : m + 1]
                    )

            scores = at.tile([1, 2048], dt.float32)
            for nn in range(4):
                ps = ap3.tile([1, 512], dt.float32, tag="sc")
                for m in range(4):
                    nc.tensor.matmul(
                        ps[:],
                        vT[:, m : m + 1],
                        u[:, m, nn * 512 : (nn + 1) * 512],
                        start=(m == 0),
                        stop=(m == 3),
                    )
                nc.vector.tensor_copy(scores[:, nn * 512 : (nn + 1) * 512], ps[:])

            scv = scores[:].rearrange("p (b t) -> p b t", t=256)
            mx = at.tile([1, 8], dt.float32)
            nc.vector.tensor_reduce(mx[:], scv, axis=AX.X, op=OP.max)
            nc.vector.tensor_tensor(
                out=scv, in0=scv, in1=mx[:].to_broadcast([1, 8, 256]), op=OP.subtract
            )
            ex = at.tile([1, 2048], dt.float32)
            nc.scalar.activation(ex[:], scores[:], AF.Exp)
            exv = ex[:].rearrange("p (b t) -> p b t", t=256)
            sm = at.tile([1, 8], dt.float32)
            nc.vector.tensor_reduce(sm[:], exv, axis=AX.X, op=OP.add)
            inv = at.tile([1, 8], dt.float32)
            nc.vector.reciprocal(inv[:], sm[:])
            attn = at.tile([1, 2048], dt.bfloat16)
            nc.vector.tensor_tensor(
                out=attn[:].rearrange("p (b t) -> p b t", t=256),
                in0=exv,
                in1=inv[:].to_broadcast([1, 8, 256]),
                op=OP.mult,
            )

            attn128 = at.tile([128, 2048], dt.float32)
            for nn in range(4):
                ps = ap1.tile([128, 512], dt.float32, tag="big")
                nc.tensor.matmul(
                    ps[:],
                    ones[:],
                    attn[:, nn * 512 : (nn + 1) * 512],
                    start=True,
                    stop=True,
                )
                nc.vector.tensor_copy(attn128[:, nn * 512 : (nn + 1) * 512], ps[:])

            ctxf = at.tile([128, KT, BL], dt.float32)
            av = attn128[:].rearrange("p (b t) -> p b t", t=256)
            for k in range(KT):
                tmp = atm.tile([128, 2048], dt.float32, tag="ctx")
                tv = tmp[:].rearrange("p (b t) -> p b t", t=256)
                hvk = hv[:, k, :].rearrange("p (t b) -> p b t", b=8)
                nc.vector.tensor_tensor(out=tv, in0=hvk, in1=av, op=OP.mult)
                nc.vector.tensor_reduce(ctxf[:, k, :], tv, axis=AX.X, op=OP.add)
            ctx = at.tile([128, KT * BL], dt.bfloat16)
            nc.vector.tensor_copy(
                ctx[:].rearrange("p (k b) -> p k b", b=8), ctxf[:]
            )

            nT = at.tile([128, 8, BL], dt.bfloat16)
            for m in range(8):
                ps = ap2.tile([128, BL], dt.float32, tag="sm")
                for k in range(16):
                    rhs = (
                        ctx[:, (k * BL) : (k * BL + BL)]
                        if k < 8
                        else h_seq[:, k - 8, hl : hl + BL]
                    )
                    nc.tensor.matmul(
                        ps[:],
                        nw[:, m * 16 + k, :],
                        rhs,
                        start=(k == 0),
                        stop=(k == 15),
                    )
                nc.scalar.activation(
                    nT[:, m, :], ps[:], AF.Tanh, bias=nb[:, m : m + 1]
                )

            psl = ap3.tile([2, BL], dt.float32, tag="sc")
            for k in range(8):
                nc.tensor.matmul(
                    psl[:],
                    ow[:, k, :],
                    nT[:, k, :],
                    start=(k == 0),
                    stop=(k == 7),
                )
            lg = at.tile([2, BL], dt.float32)
            nc.scalar.activation(lg[:], psl[:], AF.Identity, bias=ob[:])
            nc.sync.dma_start(out_d[:], lg[:])

    st.close()
    _split_waits(nc)
    lower_extended_insts(nc)
    return nc


def program_and_inmaps(inputs):
    if "nc" not in _CACHE:
        _CACHE["nc"] = _build_program()
    nc = _CACHE["nc"]

    sh = _prep_shared(inputs)
    ekey = (id(inputs["x"]), id(inputs["embed_w"]))
    ehit = _CACHE.get("embT")
    if ehit is not None and ehit[0] == ekey:
        embT = ehit[1]
    else:
        x = np.asarray(inputs["x"]).astype(np.int64)
        emb32 = _f32(inputs["embed_w"])
        xf = np.ascontiguousarray(x.T).reshape(-1)     # t-major (t*64+b)
        g = emb32[xf]                                  # [16384, 512]
        embT = _bf16(
            np.ascontiguousarray(g.T).reshape(KT0, 128, T * FB).transpose(1, 0, 2)
        )                                              # [128, 4, 16384]
        _CACHE["embT"] = (ekey, embT)

    in_maps = []
    for c in range(NCORES):
        d = sh["percore"][c]
        m = {
            "embT": embT,
            "wi0": d["wi0"], "wh0": d["wh0"], "b0": d["b0"],
            "wi1": d["wi1"], "wh1": d["wh1"], "b1": d["b1"],
            "m1w": sh["m1w"], "m2w": sh["m2w"],
            "m1b": sh["m1b"], "m2b": sh["m2b"],
            "vT": sh["vT"], "nw": sh["nw"], "nb": sh["nb"],
            "ow": sh["ow"], "ob": sh["ob"],
        }
        in_maps.append(m)
    return nc, in_maps


def kernel(**inputs):
    from concourse import bass_utils

    nc, in_maps = program_and_inmaps(inputs)
    res = bass_utils.run_bass_kernel_spmd(nc, in_maps, core_ids=list(range(NCORES)))
    out = np.zeros((B, C), np.float32)
    for c in range(NCORES):
        out[c * BL : (c + 1) * BL] = res.results[c]["out_t"].T
    return out


# revision 7
# speedup vs baseline: 1.0690x; 1.0690x over previous
"""AttentionLSTM Trainium2 kernel, v3: 8-way tensor parallel recurrence.

Each core owns 512 of the 4096 gate rows (the 4 gates g,i,f,o of a 128-dim
h-slice) for the FULL batch of 64. Per timestep a core computes its h-chunk
[128, 64] and broadcasts it to all 8 cores via remote_dma_broadcast with a
partition_id-indexed slot (sigma-independent); the next step's matmuls
consume the assembled h [8k x 64b] from the receive ring.

This cuts the per-step LDWEIGHTS wall 8x vs data-parallel (32 weight tiles
per step instead of 256), which is the dominant cost at batch-8-per-core.

Phases:
  P1 (Tile) L0 input projection from full-batch embeddings -> xp SBUF
  R0 (raw)  L0 recurrence, 256 steps, h -> h1seq DRAM
  P3 (Tile) L1 input projection from h1seq -> xp SBUF
  R1 (raw)  L1 recurrence; per-step DVE extracts this core's batch slice
            into h2sel (t-major, 8b per t — same layout the attention uses)
  P5 (Tile) additive attention + head for this core's 8 batch rows
"""

import os
import numpy as np
from contextlib import ExitStack

B, T, V, E, H, A, C = 64, 256, 32000, 512, 1024, 512, 2
NCORES = 8
BL = 8            # batch rows per core for the attention phase
FB = 64           # full batch (recurrence free dim)
MT = 4            # m-tiles per core (4 gates x 128 rows)
KT = 8            # k-tiles over H
KT0 = 4           # k-tiles over E
GPERM = [2, 0, 1, 3]   # gate row reorder: [g, i, f, o]
SEQ = (T + 1) * BL
RING = 4

_CACHE = {}


def _bf16(a):
    import ml_dtypes
    return np.ascontiguousarray(np.asarray(a, np.float32)).astype(ml_dtypes.bfloat16)


def _f32(a):
    return np.ascontiguousarray(a, np.float32)


def _tiles(w, mt, kt):
    m4 = w.reshape(mt, 128, kt, 128)
    return np.ascontiguousarray(np.transpose(m4, (3, 0, 2, 1))).reshape(
        128, mt * kt, 128
    )


def _gate_reorder(w):
    g = w.reshape(4, H, -1)
    return np.concatenate([g[p] for p in GPERM], 0)


def _core_rows(wr, j):
    """wr: [4H, D] gate-reordered. Core j's 512 rows: gates x h-slice j."""
    return np.concatenate(
        [wr[g * H + 128 * j : g * H + 128 * (j + 1)] for g in range(4)], 0
    )


def _prep_shared(inputs):
    wkeys = ("w_ih0", "w_hh0", "b_ih0", "b_hh0", "w_ih1", "w_hh1",
             "b_ih1", "b_hh1", "m1_w", "m1_b", "m2_w", "m2_b", "v",
             "n_w", "n_b", "out_w", "out_b")
    key = tuple(id(inputs[k]) for k in wkeys)
    hit = _CACHE.get("sh")
    if hit is not None and hit[0] == key:
        return hit[1]
    sh = {"percore": []}
    for L, (wi, wh, bi, bh, kt) in {
        0: ("w_ih0", "w_hh0", "b_ih0", "b_hh0", KT0),
        1: ("w_ih1", "w_hh1", "b_ih1", "b_hh1", KT),
    }.items():
        sh[f"wir{L}"] = _gate_reorder(_f32(inputs[wi]))
        sh[f"whr{L}"] = _gate_reorder(_f32(inputs[wh]))
        sh[f"br{L}"] = _gate_reorder(
            (_f32(inputs[bi]) + _f32(inputs[bh]))[:, None]
        )[:, 0]
    for j in range(NCORES):
        d = {}
        for L, kt in ((0, KT0), (1, KT)):
            d[f"wi{L}"] = _bf16(_tiles(_core_rows(sh[f"wir{L}"], j), MT, kt))
            d[f"wh{L}"] = _bf16(_tiles(_core_rows(sh[f"whr{L}"], j), MT, KT))
            d[f"b{L}"] = _f32(
                _core_rows(sh[f"br{L}"][:, None], j).reshape(MT, 128).T
            )  # [128, 4]
        sh["percore"].append(d)

    sh["m1w"] = _bf16(_tiles(_f32(inputs["m1_w"]), 4, KT))
    sh["m2w"] = _bf16(_tiles(_f32(inputs["m2_w"]), 4, KT))
    sh["m1b"] = _f32(_f32(inputs["m1_b"]).reshape(4, 128).T)
    sh["m2b"] = _f32(_f32(inputs["m2_b"]).reshape(4, 128).T)
    sh["vT"] = _bf16(_f32(inputs["v"]).reshape(4, 128).T)
    sh["nw"] = _bf16(_tiles(_f32(inputs["n_w"]), 8, 16))
    sh["nb"] = _f32(_f32(inputs["n_b"]).reshape(8, 128).T)
    ow = _f32(inputs["out_w"]).T
    sh["ow"] = _bf16(ow.reshape(8, 128, 2).transpose(1, 0, 2))
    sh["ob"] = _f32(_f32(inputs["out_b"]).reshape(2, 1))
    _CACHE["sh"] = (key, sh)
    return sh


def _install_drain_patch():
    from concourse.tile import TileContext, ScopedClock

    if getattr(TileContext, "_drain_patched", False):
        return

    def _patched(self, tick_clock, wait_clock):
        drain_inst = self.nc.sync.drain()
        wait_clock.add_sem_waits(
            drain_inst.ins, ScopedClock({None: tick_clock.global_clock})
        )
        si = drain_inst.ins.sync_info
        waits = list(si.on_wait)
        if len(waits) > 1:
            si.on_wait = waits[:1]
            for w in waits[1:]:
                d2 = self.nc.sync.drain()
                wait_clock.add_sem_waits(
                    d2.ins, ScopedClock({None: tick_clock.global_clock})
                )
                d2.ins.sync_info.on_wait = [w]
        self.nc.all_engine_barrier()
        popped = self.nc._tile_sem_poison_stack.pop()
        assert popped is self._sem_poison
        self.nc.clear_and_free_semaphores(list(self.sems.allocated().values()))
        self.nc.all_engine_barrier()

    TileContext._drain_and_barrier = _patched
    TileContext._drain_patched = True


def _split_waits(nc, limit=1):
    import copy
    import concourse.mybir as mybir

    n_split = 0
    for f in nc.m.functions:
        for bb in f.blocks:
            li = list(bb.instructions)
            out = []
            for inst in li:
                si = inst.sync_info
                if si is not None and len(si.on_wait) > limit:
                    waits = list(si.on_wait)
                    for j in range(0, len(waits) - limit, limit):
                        nop = mybir.InstNoOp(
                            name=f"{inst.name}_ws{j}", ins=[], outs=[]
                        )
                        nop.engine = inst.engine
                        si2 = copy.copy(si)
                        si2.on_wait = waits[j : j + limit]
                        si2.on_update = []
                        nop.sync_info = si2
                        out.append(nop)
                        n_split += 1
                    si.on_wait = waits[len(waits) - limit :]
                inst.sync_info = si
                out.append(inst)
            bb.instructions = out
    return n_split


def _build_program():
    import concourse.bass as bass
    import concourse.mybir as mybir
    from concourse.bass import ds
    from concourse.tile import TileContext
    from concourse import library_config
    from concourse.library_overlay import lower_extended_insts

    _install_drain_patch()

    dt = mybir.dt
    AF = mybir.ActivationFunctionType
    OP = mybir.AluOpType
    AX = mybir.AxisListType

    nc = bass.Bass(num_devices=NCORES)

    # ---- DRAM I/O ----
    embT_d = nc.dram_tensor("embT", [128, KT0, T * FB], dt.bfloat16, kind="ExternalInput")
    wi0_d = nc.dram_tensor("wi0", [128, MT * KT0, 128], dt.bfloat16, kind="ExternalInput")
    wh0_d = nc.dram_tensor("wh0", [128, MT * KT, 128], dt.bfloat16, kind="ExternalInput")
    wi1_d = nc.dram_tensor("wi1", [128, MT * KT, 128], dt.bfloat16, kind="ExternalInput")
    wh1_d = nc.dram_tensor("wh1", [128, MT * KT, 128], dt.bfloat16, kind="ExternalInput")
    b0_d = nc.dram_tensor("b0", [128, MT], dt.float32, kind="ExternalInput")
    b1_d = nc.dram_tensor("b1", [128, MT], dt.float32, kind="ExternalInput")
    m1w_d = nc.dram_tensor("m1w", [128, 4 * KT, 128], dt.bfloat16, kind="ExternalInput")
    m2w_d = nc.dram_tensor("m2w", [128, 4 * KT, 128], dt.bfloat16, kind="ExternalInput")
    m1b_d = nc.dram_tensor("m1b", [128, 4], dt.float32, kind="ExternalInput")
    m2b_d = nc.dram_tensor("m2b", [128, 4], dt.float32, kind="ExternalInput")
    vT_d = nc.dram_tensor("vT", [128, 4], dt.bfloat16, kind="ExternalInput")
    nw_d = nc.dram_tensor("nw", [128, 8 * 16, 128], dt.bfloat16, kind="ExternalInput")
    nb_d = nc.dram_tensor("nb", [128, 8], dt.float32, kind="ExternalInput")
    ow_d = nc.dram_tensor("ow", [128, 8, 2], dt.bfloat16, kind="ExternalInput")
    ob_d = nc.dram_tensor("ob", [2, 1], dt.float32, kind="ExternalInput")
    out_d = nc.dram_tensor("out_t", [2, BL], dt.float32, kind="ExternalOutput")
    h1seq_d = nc.dram_tensor("h1seq", [128, T, KT * FB], dt.bfloat16, kind="Internal")
    xpdump_d = None

    st = ExitStack()
    # ---- persistent SBUF ----
    wh = st.enter_context(nc.sbuf_tensor("wh", [128, MT * KT, 128], dt.bfloat16))
    recv = st.enter_context(nc.sbuf_tensor("recv", [128, RING, KT * FB], dt.bfloat16))
    send = st.enter_context(nc.sbuf_tensor("send", [128, FB], dt.bfloat16))
    h2sel = st.enter_context(nc.sbuf_tensor("h2sel", [128, KT, SEQ], dt.bfloat16))
    cst = st.enter_context(nc.sbuf_tensor("cst", [128, FB], dt.float32))
    gsum = st.enter_context(nc.sbuf_tensor("gsum", [128, 4, FB], dt.float32))
    gact = st.enter_context(nc.sbuf_tensor("gact", [128, 4, FB], dt.float32))
    t1 = st.enter_context(nc.sbuf_tensor("t1", [128, FB], dt.float32))
    c2 = st.enter_context(nc.sbuf_tensor("c2", [128, FB], dt.float32))
    tcs = st.enter_context(nc.sbuf_tensor("tcs", [128, FB], dt.float32))
    dummy = st.enter_context(nc.sbuf_tensor("dmy_s", [128, 1], dt.float32))
    # xp is huge (128KB/partition); allocated on its own stack so it can be
    # freed before the attention phase needs the space.
    stx = ExitStack()
    xp = stx.enter_context(nc.sbuf_tensor("xp", [128, MT, T * FB], dt.bfloat16))

    # ---- semaphores (raw sections) ----
    sem = {}
    for L in range(2):
        for nm in ("lsem", "psem", "hsem", "mmsem", "actsem",
                   "dvasem", "dvcsem"):
            sem[f"{nm}{L}"] = nc.alloc_semaphore(f"{nm}{L}")
        sem[f"rsems{L}"] = [
            nc.alloc_semaphore(f"rsem{L}_{q}") for q in range(RING)
        ]
    whdsem = nc.alloc_semaphore("whdsem")
    h1dsem = nc.alloc_semaphore("h1dsem")

    # gpsimd prologue: comm library + identity registers
    nc.gpsimd.load_library(library_config.remote_dma)
    pid_g = nc.gpsimd.partition_id()
    pid_v = nc.vector.partition_id()
    pg64 = pid_g * FB
    pv8 = pid_v * BL

    # initial wh0 load (SP)
    nc.sync.dma_start(wh[:], wh0_d[:]).then_inc(whdsem, 16)

    # =========== P1: L0 input projection (Tile) ===========
    with TileContext(nc) as tc:
        with (
            tc.tile_pool(name="p1", bufs=1) as p1,
            tc.tile_pool(name="emb", bufs=3) as ep,
            tc.tile_pool(name="pp1", bufs=4, space="PSUM") as pp1,
        ):
            wi0 = p1.tile([128, MT * KT0, 128], dt.bfloat16)
            nc.sync.dma_start(wi0[:], wi0_d[:])
            b0s = p1.tile([128, MT], dt.float32)
            nc.sync.dma_start(b0s[:], b0_d[:])
            nc.vector.memset(h2sel[:, :, 0:BL], 0.0)
            nc.vector.memset(cst[:], 0.0)
            for nn in range(32):
                eb = ep.tile([128, KT0, 512], dt.bfloat16, tag="eb")
                nc.sync.dma_start(eb[:], embT_d[:, :, nn * 512 : (nn + 1) * 512])
                for m in range(MT):
                    ps = pp1.tile([128, 512], dt.float32, tag="mm")
                    for k in range(KT0):
                        nc.tensor.matmul(
                            ps[:],
                            wi0[:, m * KT0 + k, :],
                            eb[:, k, :],
                            start=(k == 0),
                            stop=(k == KT0 - 1),
                        )
                    nc.scalar.activation(
                        xp[:, m, nn * 512 : (nn + 1) * 512],
                        ps[:],
                        AF.Identity,
                        bias=b0s[:, m : m + 1],
                    )

    # =========== recurrence (raw) ===========
    ctr = dict()

    def inc(s):
        ctr[s] = ctr.get(s, 0) + 1
        return ctr[s]

    def recurrence(L, extract):
        rsems, lsem, psem = sem[f"rsems{L}"], sem[f"lsem{L}"], sem[f"psem{L}"]

        def rwait(eng, rnd):
            # arrivals of round `rnd` complete: its ring-slot sem counts
            # 16 per use of that slot
            eng.wait_ge(rsems[rnd % RING], 16 * (rnd // RING + 1))
        hsem, mmsem = sem[f"hsem{L}"], sem[f"mmsem{L}"]
        actsem, dvasem, dvcsem = (
            sem[f"actsem{L}"], sem[f"dvasem{L}"], sem[f"dvcsem{L}"],
        )
        rst = ExitStack()
        PS = [
            rst.enter_context(
                nc.psum_tensor(f"ps{L}_{m}", [128, FB], dt.float32)
            )
            for m in range(4)
        ]
        # drain scratch: the per-gate "done" sem is deferred to the NEXT
        # group's stop matmul so the PSUM writeback pipeline (128-cycle
        # drain) is provably flushed before the DVE reads the bank. The
        # last gate is covered by a 3-matmul dummy group into PSX.
        PSX = rst.enter_context(
            nc.psum_tensor(f"psx{L}", [128, FB], dt.float32)
        )

        # keep both ACT tables resident before the loop
        nc.scalar.activation(dummy[:], dummy[:], AF.Tanh)
        nc.scalar.activation(dummy[:], dummy[:], AF.Sigmoid)

        def gen_desc(r):
            nc.gpsimd.remote_dma_broadcast(
                recv[:, r % RING, :][:, ds(pg64, FB)],
                send[:],
                remote_sem=rsems[r % RING],
                local_sem=lsem,
                rdests=[(0, k) for k in range(NCORES)],
            ).then_inc(psem, 1)
            inc(psem.name)

        def xs(m, r):
            return xp[:, m, r * FB : (r + 1) * FB]

        # ---- step 0: gates = xs (h_{-1}=0); c0 = sig(i)*tanh(g) ----
        a_tg = inc(actsem.name)
        nc.scalar.activation(gact[:, 0, :], xs(0, 0), AF.Tanh).then_inc(actsem, 1)
        a_si = inc(actsem.name)
        nc.scalar.activation(gact[:, 1, :], xs(1, 0), AF.Sigmoid).then_inc(actsem, 1)
        a_so = inc(actsem.name)
        nc.scalar.activation(gact[:, 3, :], xs(3, 0), AF.Sigmoid).then_inc(actsem, 1)
        nc.vector.wait_ge(actsem, a_si)
        nc.vector.tensor_tensor(
            out=cst[:], in0=gact[:, 1, :], in1=gact[:, 0, :], op=OP.mult
        ).then_inc(dvcsem, 1)
        inc(dvcsem.name)
        a_tc = inc(actsem.name)
        nc.scalar.wait_ge(dvcsem, ctr[dvcsem.name])
        nc.scalar.activation(tcs[:], cst[:], AF.Tanh).then_inc(actsem, 1)
        nc.vector.wait_ge(actsem, a_tc)
        nc.vector.tensor_tensor(
            out=send[:], in0=gact[:, 3, :], in1=tcs[:], op=OP.mult
        ).then_inc(hsem, 1)
        inc(hsem.name)
        gen_desc(0)

        # gpsimd comm schedule for all rounds
        for r in range(T):
            # psem first (satisfied a round in advance); final wake on hsem
            nc.gpsimd.wait_ge(psem, r + 1)
            nc.gpsimd.wait_ge(hsem, r + 1)
            nc.gpsimd.trigger_dma(count=1)
            if r + 1 < T:
                gen_desc(r + 1)

        # ---- steps 1..T-1 ----
        for r in range(1, T):
            rv = recv[:, (r - 1) % RING, :]
            # PE
            rwait(nc.tensor, r - 1)
            for m in range(4):
                for k in range(KT):
                    mm = nc.tensor.matmul(
                        PS[m][:],
                        wh[:, m * KT + k, :],
                        rv[:, k * FB : (k + 1) * FB],
                        start=(k == 0),
                        stop=(k == KT - 1),
                    )
                if m >= 1:
                    # group m's stop also certifies group m-1's writeback
                    mm.then_inc(mmsem, 1)
                    inc(mmsem.name)
            for k in range(3):
                mm = nc.tensor.matmul(
                    PSX[:],
                    wh[:, 3 * KT + KT - 1, :],
                    rv[:, (KT - 1) * FB : KT * FB],
                    start=(k == 0),
                    stop=(k == 2),
                )
            mm.then_inc(mmsem, 1)
            inc(mmsem.name)
            mmbase = ctr[mmsem.name] - 4

            # DVE: extraction of round r-1 data (L1 only)
            if extract:
                rwait(nc.vector, r - 1)
                nc.vector.tensor_copy(
                    h2sel[:, :, r * BL : (r + 1) * BL],
                    rv.rearrange("p (k b) -> p k b", b=FB)[:, :, ds(pv8, BL)],
                )

            # DVE adds (gate = psum + xs)
            for m in range(4):
                nc.vector.wait_ge(mmsem, mmbase + m + 1)
                nc.vector.tensor_tensor(
                    out=gsum[:, m, :], in0=PS[m][:], in1=xs(m, r), op=OP.add
                ).then_inc(dvasem, inc(dvasem.name))
            dvabase = ctr[dvasem.name] - 4

            # ACT nonlinearities
            acts = []
            for m, f in ((0, AF.Tanh), (1, AF.Sigmoid), (2, AF.Sigmoid),
                         (3, AF.Sigmoid)):
                nc.scalar.wait_ge(dvasem, dvabase + m + 1)
                nc.scalar.activation(gact[:, m, :], gsum[:, m, :], f).then_inc(
                    actsem, 1
                )
                acts.append(inc(actsem.name))

            # DVE cell chain
            nc.vector.wait_ge(actsem, acts[1])
            nc.vector.tensor_tensor(
                out=t1[:], in0=gact[:, 1, :], in1=gact[:, 0, :], op=OP.mult
            )
            nc.vector.wait_ge(actsem, acts[2])
            nc.vector.tensor_tensor(
                out=c2[:], in0=gact[:, 2, :], in1=cst[:], op=OP.mult
            )
            nc.vector.tensor_tensor(
                out=cst[:], in0=c2[:], in1=t1[:], op=OP.add
            ).then_inc(dvcsem, 1)
            inc(dvcsem.name)
            a_tc = inc(actsem.name)
            nc.scalar.wait_ge(dvcsem, ctr[dvcsem.name])
            nc.scalar.activation(tcs[:], cst[:], AF.Tanh).then_inc(actsem, 1)
            # h = sig(o) * tanh(c)  -> send buffer (bf16)
            # lsem guard IS required: the arrival gate only certifies the
            # self-loopback lanes; the 14 remote-dest lanes' reads of `send`
            # are certified only by lsem (+16 when all lanes sent).
            nc.vector.wait_ge(actsem, a_tc)
            nc.vector.wait_ge(lsem, 16 * r)
            nc.vector.tensor_tensor(
                out=send[:], in0=gact[:, 3, :], in1=tcs[:], op=OP.mult
            ).then_inc(hsem, 1)
            inc(hsem.name)

            # SP: stream assembled h of round r-1 out (L0 only)
            if not extract:
                rwait(nc.sync, r - 1)
                nc.sync.dma_start(h1seq_d[:, r - 1, :], rv[:]).then_inc(h1dsem, 16)
                inc("h1d")

        # tail: last round's assembled h
        if extract:
            rwait(nc.vector, T - 1)
            nc.vector.tensor_copy(
                h2sel[:, :, T * BL : (T + 1) * BL],
                recv[:, (T - 1) % RING, :].rearrange("p (k b) -> p k b", b=FB)[
                    :, :, ds(pv8, BL)
                ],
            )
        else:
            rwait(nc.sync, T - 1)
            nc.sync.dma_start(
                h1seq_d[:, T - 1, :], recv[:, (T - 1) % RING, :]
            ).then_inc(h1dsem, 16)
            inc("h1d")
            nc.sync.wait_ge(h1dsem, 16 * ctr["h1d"])
        rst.close()

    if xpdump_d is not None:
        xpd_sem = nc.alloc_semaphore("xpd_sem")
        nc.sync.dma_start(xpdump_d[:], xp[:]).then_inc(xpd_sem, 16)
        nc.sync.wait_ge(xpd_sem, 16)
        nc.all_engine_barrier()

    # ---- R0: L0 recurrence ----
    nc.tensor.wait_ge(whdsem, 16)
    recurrence(0, extract=False)
    nc.all_engine_barrier()

    # load wh1 over wh
    nc.sync.dma_start(wh[:], wh1_d[:]).then_inc(whdsem, 16)

    # =========== P3: L1 input projection (Tile) ===========
    with TileContext(nc) as tc:
        with (
            tc.tile_pool(name="p3", bufs=1) as p3,
            tc.tile_pool(name="h1b", bufs=2) as hp,
            tc.tile_pool(name="pp3", bufs=4, space="PSUM") as pp3,
        ):
            wi1 = p3.tile([128, MT * KT, 128], dt.bfloat16)
            nc.sync.dma_start(wi1[:], wi1_d[:])
            b1s = p3.tile([128, MT], dt.float32)
            nc.sync.dma_start(b1s[:], b1_d[:])
            for nn in range(32):
                hb = hp.tile([128, 8, KT * FB], dt.bfloat16, tag="hb")
                nc.sync.dma_start(hb[:], h1seq_d[:, nn * 8 : (nn + 1) * 8, :])
                for m in range(MT):
                    ps = pp3.tile([128, 512], dt.float32, tag="mm")
                    psv = ps[:].rearrange("p (t b) -> p t b", b=FB)
                    for k in range(KT):
                        nc.tensor.matmul(
                            psv,
                            wi1[:, m * KT + k, :],
                            hb[:, :, k * FB : (k + 1) * FB],
                            start=(k == 0),
                            stop=(k == KT - 1),
                        )
                    nc.scalar.activation(
                        xp[:, m, nn * 512 : (nn + 1) * 512],
                        ps[:],
                        AF.Identity,
                        bias=b1s[:, m : m + 1],
                    )

    # ---- R1: L1 recurrence ----
    nc.tensor.wait_ge(whdsem, 32)
    recurrence(1, extract=True)
    nc.all_engine_barrier()
    stx.close()

    # =========== P5: attention + head (Tile) ===========
    h_seq = h2sel
    with TileContext(nc) as tc:
        with (
            tc.tile_pool(name="att", bufs=1) as at,
            tc.tile_pool(name="attm", bufs=2) as atm,
            tc.tile_pool(name="ap1", bufs=2, space="PSUM") as ap1,
            tc.tile_pool(name="ap2", bufs=2, space="PSUM") as ap2,
            tc.tile_pool(name="ap3", bufs=1, space="PSUM") as ap3,
        ):
            m1w = at.tile([128, 4 * KT, 128], dt.bfloat16)
            nc.sync.dma_start(m1w[:], m1w_d[:])
            m2w = at.tile([128, 4 * KT, 128], dt.bfloat16)
            nc.sync.dma_start(m2w[:], m2w_d[:])
            m1b = at.tile([128, 4], dt.float32)
            nc.sync.dma_start(m1b[:], m1b_d[:])
            m2b = at.tile([128, 4], dt.float32)
            nc.sync.dma_start(m2b[:], m2b_d[:])
            vT = at.tile([128, 4], dt.bfloat16)
            nc.sync.dma_start(vT[:], vT_d[:])
            nw = at.tile([128, 8 * 16, 128], dt.bfloat16)
            nc.sync.dma_start(nw[:], nw_d[:])
            nb = at.tile([128, 8], dt.float32)
            nc.sync.dma_start(nb[:], nb_d[:])
            ow = at.tile([128, 8, 2], dt.bfloat16)
            nc.sync.dma_start(ow[:], ow_d[:])
            ob = at.tile([2, 1], dt.float32)
            nc.sync.dma_start(ob[:], ob_d[:])
            ones = at.tile([1, 128], dt.bfloat16)
            nc.vector.memset(ones[:], 1.0)

            hl = T * BL
            hv = h_seq[:, :, BL:SEQ]

            m2T = at.tile([128, 4, BL], dt.float32)
            for m in range(4):
                ps = ap2.tile([128, BL], dt.float32, tag="sm")
                for k in range(KT):
                    nc.tensor.matmul(
                        ps[:],
                        m2w[:, m * KT + k, :],
                        h_seq[:, k, hl : hl + BL],
                        start=(k == 0),
                        stop=(k == KT - 1),
                    )
                nc.scalar.activation(
                    m2T[:, m, :], ps[:], AF.Identity, bias=m2b[:, m : m + 1]
                )

            u = at.tile([128, 4, 2048], dt.bfloat16)
            tmpu = atm.tile([128, 512], dt.float32, tag="tmpu")
            for m in range(4):
                for nn in range(4):
                    t0 = nn * 64
                    ps = ap1.tile([128, 512], dt.float32, tag="big")
                    psv = ps[:].rearrange("p (b t) -> p b t", t=64)
                    for k in range(KT):
                        rhs = (
                            hv[:, k, :]
                            .rearrange("p (t b) -> p b t", b=8)[
                                :, :, t0 : t0 + 64
                            ]
                        )
                        nc.tensor.matmul(
                            psv,
                            m1w[:, m * KT + k, :],
                            rhs,
                            start=(k == 0),
                            stop=(k == KT - 1),
                        )
                    tv = tmpu[:].rearrange("p (b t) -> p b t", t=64)
                    nc.vector.tensor_tensor(
                        out=tv,
                        in0=psv,
                        in1=m2T[:, m, :].to_broadcast([128, 8, 64]),
                        op=OP.add,
                    )
                    uv = u[:, m, :].rearrange("p (b t) -> p b t", t=256)[
                        :, :, t0 : t0 + 64
                    ]
                    nc.scalar.activation(
                        uv, tv, AF.Tanh, bias=m1b[:, m : m + 1]
                    )

            scores = at.tile([1, 2048], dt.float32)
            for nn in range(4):
                ps = ap3.tile([1, 512], dt.float32, tag="sc")
                for m in range(4):
                    nc.tensor.matmul(
                        ps[:],
                        vT[:, m : m + 1],
                        u[:, m, nn * 512 : (nn + 1) * 512],
                        start=(m == 0),
                        stop=(m == 3),
                    )
                nc.vector.tensor_copy(scores[:, nn * 512 : (nn + 1) * 512], ps[:])

            scv = scores[:].rearrange("p (b t) -> p b t", t=256)
            mx = at.tile([1, 8], dt.float32)
            nc.vector.tensor_reduce(mx[:], scv, axis=AX.X, op=OP.max)
            nc.vector.tensor_tensor(
                out=scv, in0=scv, in1=mx[:].to_broadcast([1, 8, 256]), op=OP.subtract
            )
            ex = at.tile([1, 2048], dt.float32)
            nc.scalar.activation(ex[:], scores[:], AF.Exp)
            exv = ex[:].rearrange("p (b t) -> p b t", t=256)
            sm = at.tile([1, 8], dt.float32)
            nc.vector.tensor_reduce(sm[:], exv, axis=AX.X, op=OP.add)
            inv = at.tile([1, 8], dt.float32)
            nc.vector.reciprocal(inv[:], sm[:])
            attn = at.tile([1, 2048], dt.bfloat16)
            nc.vector.tensor_tensor(
                out=attn[:].rearrange("p (b t) -> p b t", t=256),
                in0=exv,
                in1=inv[:].to_broadcast([1, 8, 256]),
                op=OP.mult,
            )

            attn128 = at.tile([128, 2048], dt.float32)
            for nn in range(4):
                ps = ap1.tile([128, 512], dt.float32, tag="big")
                nc.tensor.matmul(
                    ps[:],
                    ones[:],
                    attn[:, nn * 512 : (nn + 1) * 512],
                    start=True,
                    stop=True,
                )
                nc.vector.tensor_copy(attn128[:, nn * 512 : (nn + 1) * 512], ps[:])

            ctxf = at.tile([128, KT, BL], dt.float32)
            av = attn128[:].rearrange("p (b t) -> p b t", t=256)
            for k in range(KT):
                tmp = atm.tile([128, 2048], dt.float32, tag="ctx")
                tv = tmp[:].rearrange("p (b t) -> p b t", t=256)
                hvk = hv[:, k, :].rearrange("p (t b) -> p b t", b=8)
                nc.vector.tensor_tensor(out=tv, in0=hvk, in1=av, op=OP.mult)
                nc.vector.tensor_reduce(ctxf[:, k, :], tv, axis=AX.X, op=OP.add)
            ctx = at.tile([128, KT * BL], dt.bfloat16)
            nc.vector.tensor_copy(
                ctx[:].rearrange("p (k b) -> p k b", b=8), ctxf[:]
            )

            nT = at.tile([128, 8, BL], dt.bfloat16)
            for m in range(8):
                ps = ap2.tile([128, BL], dt.float32, tag="sm")
                for k in range(16):
                    rhs = (
                        ctx[:, (k * BL) : (k * BL + BL)]
                        if k < 8
                        else h_seq[:, k - 8, hl : hl + BL]
                    )
                    nc.tensor.matmul(
                        ps[:],
                        nw[:, m * 16 + k, :],
                        rhs,
                        start=(k == 0),
                        stop=(k == 15),
                    )
                nc.scalar.activation(
                    nT[:, m, :], ps[:], AF.Tanh, bias=nb[:, m : m + 1]
                )

            psl = ap3.tile([2, BL], dt.float32, tag="sc")
            for k in range(8):
                nc.tensor.matmul(
                    psl[:],
                    ow[:, k, :],
                    nT[:, k, :],
                    start=(k == 0),
                    stop=(k == 7),
                )
            lg = at.tile([2, BL], dt.float32)
            nc.scalar.activation(lg[:], psl[:], AF.Identity, bias=ob[:])
            nc.sync.dma_start(out_d[:], lg[:])

    st.close()
    _split_waits(nc)
    lower_extended_insts(nc)
    return nc


def program_and_inmaps(inputs):
    if "nc" not in _CACHE:
        _CACHE["nc"] = _build_program()
    nc = _CACHE["nc"]

    sh = _prep_shared(inputs)
    ekey = (id(inputs["x"]), id(inputs["embed_w"]))
    ehit = _CACHE.get("embT")
    if ehit is not None and ehit[0] == ekey:
        embT = ehit[1]
    else:
        x = np.asarray(inputs["x"]).astype(np.int64)
        emb32 = _f32(inputs["embed_w"])
        xf = np.ascontiguousarray(x.T).reshape(-1)     # t-major (t*64+b)
        g = emb32[xf]                                  # [16384, 512]
        embT = _bf16(
            np.ascontiguousarray(g.T).reshape(KT0, 128, T * FB).transpose(1, 0, 2)
        )                                              # [128, 4, 16384]
        _CACHE["embT"] = (ekey, embT)

    in_maps = []
    for c in range(NCORES):
        d = sh["percore"][c]
        m = {
            "embT": embT,
            "wi0": d["wi0"], "wh0": d["wh0"], "b0": d["b0"],
            "wi1": d["wi1"], "wh1": d["wh1"], "b1": d["b1"],
            "m1w": sh["m1w"], "m2w": sh["m2w"],
            "m1b": sh["m1b"], "m2b": sh["m2b"],
            "vT": sh["vT"], "nw": sh["nw"], "nb": sh["nb"],
            "ow": sh["ow"], "ob": sh["ob"],
        }
        in_maps.append(m)
    return nc, in_maps


def kernel(**inputs):
    from concourse import bass_utils

    nc, in_maps = program_and_inmaps(inputs)
    res = bass_utils.run_bass_kernel_spmd(nc, in_maps, core_ids=list(range(NCORES)))
    out = np.zeros((B, C), np.float32)
    for c in range(NCORES):
        out[c * BL : (c + 1) * BL] = res.results[c]["out_t"].T
    return out
